# revision 10
# baseline (speedup 1.0000x reference)
"""Trainium2 Bass kernel for nn_DSSM (Mamba-like selective-scan block).

Reference math (B=4, L=4096, D=1024, ED=2048, N=16, K=3):
    proj = x @ W_in.T ; x_conv_pre, x_ssm = split(proj)
    x_conv = depthwise_conv1d(x_conv_pre, conv_w, pad=1)
    dt = mean_e(x_ssm); dtv = dt * W_dt[:,0]
    a = dtv @ A ; u = (dtv * x_ssm) @ Bm          # [b, l, N]
    m_t = a_t * m_{t-1} + u_t  (scan over l)
    y = m @ Cm + Dv * x_ssm
    z = x_conv * sig(y) + y * (1 - sig(y))
    out = z @ W_out.T + x

Algebraic folding (host, exact):
    dt = x @ w_mean              where w_mean = mean_e(W_ssm)
    a  = dt * s_a                where s_a = A.T @ W_dt[:,0]
    u  = dt * (x @ G)            where G = W_ssm.T @ (W_dt[:,0] * Bm)
    Dv folded into the ssm half of W_in (y = m@Cm + x@(Dv*W_ssm).T)

Sharding: core c -> batch c//2, L-half c%2 (2048 rows each). The scan is
seeded by a 512-row warmup for odd cores (|a| << 1 so the recurrence
forgets its initial state; 512 steps is astronomically safe). Conv
boundary columns are computed by a small standalone matmul pass.

Precision: in-proj / small GEMMs in float32r (TF32-like, ~1.5e-4),
out-proj in bf16 (z and W_out bf16), everything else fp32.
"""
import sys
sys.path.insert(0, '/opt/trn_rl_repo')

import numpy as np
import ml_dtypes

import concourse.bass as bass
import concourse.bacc as bacc
import concourse.tile as tile
import concourse.mybir as mybir
from concourse.bass_utils import run_bass_kernel_spmd

F32 = mybir.dt.float32
F32R = mybir.dt.float32r
BF16 = mybir.dt.bfloat16
MULT = mybir.AluOpType.mult
ADD = mybir.AluOpType.add
SUBT = mybir.AluOpType.subtract
SIG = mybir.ActivationFunctionType.Sigmoid

B_SZ, L, D, ED, N = 4, 4096, 1024, 2048, 16
N_CORES = 8
RPC = 2048          # rows per core
SUB = 512           # rows per sub-chunk
NSUB = RPC // SUB   # 4
WARM = 512          # scan warmup rows (one sub-chunk)
NKT = D // 128      # 8 k-tiles over the contraction dim
NET = ED // 128     # 16 e-tiles per half

# conv halo row indices relative to the core's first row: head/tail of each
# sub-chunk boundary. head(s) = HALO_HEAD[s], tail(s) = HALO_TAIL[s].
HALO_REL = [-1, 511, 512, 1023, 1024, 1535, 1536, 2048]
HALO_HEAD = [0, 1, 3, 5]
HALO_TAIL = [2, 4, 6, 7]

_CACHED_NC = None


def build_kernel():
    nc = bacc.Bacc("TRN2", target_bir_lowering=False, debug=False,
                   num_devices=N_CORES)

    X = nc.dram_tensor("x", [RPC, D], F32, kind="ExternalInput")
    XW = nc.dram_tensor("xw", [WARM, D], F32, kind="ExternalInput")
    XH = nc.dram_tensor("xh", [8, D], F32, kind="ExternalInput")
    WT = nc.dram_tensor("wt", [D, 2 * ED], F32R, kind="ExternalInput")
    WO = nc.dram_tensor("wo", [ED, D], BF16, kind="ExternalInput")
    CM = nc.dram_tensor("cm", [N, ED], BF16, kind="ExternalInput")
    HM = nc.dram_tensor("hm", [D, 17], F32R, kind="ExternalInput")
    SA = nc.dram_tensor("sa", [N, 1], F32, kind="ExternalInput")
    CW = nc.dram_tensor("cw", [NET, 128, 3], F32, kind="ExternalInput")
    IDENT = nc.dram_tensor("ident", [128, 128], F32, kind="ExternalInput")
    OUT = nc.dram_tensor("out", [RPC, D], F32, kind="ExternalOutput")

    with tile.TileContext(nc) as tc:
        with (
            tc.tile_pool(name="const", bufs=1) as cpool,
            tc.tile_pool(name="xnat", bufs=5) as xnat_pool,
            tc.tile_pool(name="xt", bufs=10) as xt_pool,
            tc.tile_pool(name="wst", bufs=8) as w_pool,
            tc.tile_pool(name="pre", bufs=4) as pre_pool,
            tc.tile_pool(name="gy", bufs=3) as gy_pool,
            tc.tile_pool(name="cvt", bufs=2) as cv_pool,
            tc.tile_pool(name="zp", bufs=18) as z_pool,
            tc.tile_pool(name="scn", bufs=2) as s_pool,
            tc.tile_pool(name="ob", bufs=3) as o_pool,
            tc.tile_pool(name="xr", bufs=3) as xr_pool,
            tc.tile_pool(name="tps", bufs=2, space="PSUM") as t_ps,
            tc.tile_pool(name="sps", bufs=2, space="PSUM") as s_ps,
            tc.tile_pool(name="fps", bufs=2, space="PSUM") as f_ps,
            tc.tile_pool(name="ops", bufs=2, space="PSUM") as o_ps,
        ):
            # ---- resident constants ----
            ident = cpool.tile([128, 128], F32, tag="ident")
            nc.sync.dma_start(ident[:], IDENT[:])
            wo_sb = cpool.tile([128, NET * D], BF16, tag="wo")
            nc.sync.dma_start(
                wo_sb[:].rearrange("p (e j) -> p e j", e=NET),
                WO[:].rearrange("(e p) j -> p e j", p=128))
            cm_sb = cpool.tile([N, ED], BF16, tag="cm")
            nc.sync.dma_start(cm_sb[:], CM[:])
            h_sb = cpool.tile([128, NKT * 17], F32R, tag="hm")
            nc.sync.dma_start(
                h_sb[:].rearrange("p (k j) -> p k j", k=NKT),
                HM[:].rearrange("(k p) j -> p k j", p=128))
            sa_sb = cpool.tile([N, 1], F32, tag="sa")
            nc.sync.dma_start(sa_sb[:], SA[:])
            cw_sb = cpool.tile([128, NET * 3], F32, tag="cw")
            nc.sync.dma_start(
                cw_sb[:].rearrange("p (i k) -> p i k", i=NET),
                CW[:].rearrange("i p k -> p i k"))
            halo_all = cpool.tile([128, NET * 8], F32, tag="halo")
            ones1 = cpool.tile([1, N], F32, tag="ones1")
            nc.vector.memset(ones1[:], 1.0)
            zero16 = cpool.tile([N, 1], F32, tag="zero16")
            nc.vector.memset(zero16[:], 0.0)

            def load_w_tile(k, e):
                wt_t = w_pool.tile([128, 128], F32R, tag="wt")
                nc.sync.dma_start(
                    wt_t[:], WT[k * 128:(k + 1) * 128, e * 128:(e + 1) * 128])
                return wt_t

            def transpose_128(dst, src_ap, pcount=128):
                """PE-transpose src_ap [p, 128] -> dst [128, p] (f32r out)."""
                pt = t_ps.tile([128, pcount], F32, tag="tps")
                nc.tensor.transpose(pt[:], src_ap, ident[0:pcount, 0:pcount])
                nc.scalar.copy(dst, pt[:])

            # ---- conv-halo prologue: pre[halo_rows, e] for all 16 e-tiles ----
            xh_nat = xnat_pool.tile([8, D], F32, tag="xnat")
            nc.sync.dma_start(xh_nat[:], XH[:])
            xth = []
            for k in range(NKT):
                t = cpool.tile([128, 8], F32R, tag=f"xth{k}")
                transpose_128(t[:], xh_nat[:, k * 128:(k + 1) * 128], pcount=8)
                xth.append(t)
            for i in range(NET):
                ph = s_ps.tile([128, 8], F32, tag="sps")
                for k in range(NKT):
                    wt_t = load_w_tile(k, i)
                    nc.tensor.matmul(ph[:], wt_t[:], xth[k][:],
                                     start=(k == 0), stop=(k == NKT - 1))
                nc.vector.tensor_copy(halo_all[:, i * 8:(i + 1) * 8], ph[:])

            # ---- per-sub-chunk pipeline ----
            prev_m = None

            def scan_path(xt_tiles, first):
                """small GEMM -> dt broadcast -> a,u -> scan. Returns m tile."""
                nonlocal prev_m
                psv = s_ps.tile([N, SUB], F32, tag="sps")
                pdt = s_ps.tile([1, SUB], F32, tag="sps")
                for k in range(NKT):
                    nc.tensor.matmul(psv[:], h_sb[:, k * 17:k * 17 + 16],
                                     xt_tiles[k][:], start=(k == 0),
                                     stop=(k == NKT - 1))
                for k in range(NKT):
                    nc.tensor.matmul(pdt[:], h_sb[:, k * 17 + 16:k * 17 + 17],
                                     xt_tiles[k][:], start=(k == 0),
                                     stop=(k == NKT - 1))
                sv = s_pool.tile([N, SUB], F32, tag="sv")
                nc.vector.tensor_copy(sv[:], psv[:])
                dtr = s_pool.tile([1, SUB], F32, tag="dtr")
                nc.vector.tensor_copy(dtr[:], pdt[:])
                pdtb = s_ps.tile([N, SUB], F32, tag="sps")
                nc.tensor.matmul(pdtb[:], ones1[:], dtr[:], start=True, stop=True)
                a_sb = s_pool.tile([N, SUB], F32, tag="a")
                nc.vector.tensor_scalar_mul(a_sb[:], pdtb[:], sa_sb[:])
                u_sb = s_pool.tile([N, SUB], F32, tag="u")
                nc.vector.tensor_mul(u_sb[:], sv[:], pdtb[:])
                m = s_pool.tile([N, SUB], F32, tag="m")
                init = zero16[:] if first else prev_m[:, SUB - 1:SUB]
                nc.vector.tensor_tensor_scan(m[:], a_sb[:], u_sb[:], init,
                                             op0=MULT, op1=ADD)
                prev_m = m
                return m

            def load_and_transpose(dram, row0):
                """4 nat tiles + 8 xT tiles [128, SUB] (f32r) for 512 rows."""
                xts = []
                nats = []
                for r in range(4):
                    nat = xnat_pool.tile([128, D], F32, tag="xnat")
                    nc.sync.dma_start(nat[:], dram[row0 + r * 128:
                                                   row0 + (r + 1) * 128, :])
                    nats.append(nat)
                for k in range(NKT):
                    xt = xt_pool.tile([128, SUB], F32R, tag="xt")
                    for r in range(4):
                        transpose_128(xt[:, r * 128:(r + 1) * 128],
                                      nats[r][:, k * 128:(k + 1) * 128])
                    xts.append(xt)
                return xts

            # warmup sub-chunk (scan only)
            xtw = load_and_transpose(XW, 0)
            scan_path(xtw, first=True)

            for s in range(NSUB):
                xts = load_and_transpose(X, s * SUB)
                m = scan_path(xts, first=False)
                m_bf = s_pool.tile([N, SUB], BF16, tag="mbf")
                nc.vector.tensor_copy(m_bf[:], m[:])

                z_tiles = []
                for i in range(NET):
                    # conv half e-tile
                    pc = f_ps.tile([128, SUB], F32, tag="fps")
                    for k in range(NKT):
                        wt_t = load_w_tile(k, i)
                        nc.tensor.matmul(pc[:], wt_t[:], xts[k][:],
                                         start=(k == 0), stop=(k == NKT - 1))
                    pre = pre_pool.tile([128, SUB + 2], F32, tag="pre")
                    nc.scalar.copy(pre[:, 1:SUB + 1], pc[:])
                    hc = i * 8 + HALO_HEAD[s]
                    tc_ = i * 8 + HALO_TAIL[s]
                    nc.vector.tensor_copy(pre[:, 0:1], halo_all[:, hc:hc + 1])
                    nc.vector.tensor_copy(pre[:, SUB + 1:SUB + 2],
                                          halo_all[:, tc_:tc_ + 1])
                    # ssm half e-tile (+ y accumulation)
                    py = f_ps.tile([128, SUB], F32, tag="fps")
                    for k in range(NKT):
                        wt_t = load_w_tile(k, NET + i)
                        nc.tensor.matmul(py[:], wt_t[:], xts[k][:],
                                         start=(k == 0), stop=False)
                    nc.tensor.matmul(py[:], cm_sb[:, i * 128:(i + 1) * 128],
                                     m_bf[:], start=False, stop=True)
                    g = gy_pool.tile([128, SUB], F32, tag="g")
                    nc.scalar.activation(g[:], py[:], SIG)
                    ysb = gy_pool.tile([128, SUB], F32, tag="ysb")
                    nc.vector.tensor_copy(ysb[:], py[:])
                    # conv + gate:  w = conv(pre) - y ; z = y + sig(y) * w
                    w0 = cw_sb[:, i * 3 + 0:i * 3 + 1]
                    w1 = cw_sb[:, i * 3 + 1:i * 3 + 2]
                    w2 = cw_sb[:, i * 3 + 2:i * 3 + 3]
                    s1 = cv_pool.tile([128, SUB], F32, tag="s1")
                    nc.vector.scalar_tensor_tensor(s1[:], pre[:, 1:SUB + 1],
                                                   w1, ysb[:], op0=MULT, op1=SUBT)
                    s2 = cv_pool.tile([128, SUB], F32, tag="s2")
                    nc.vector.scalar_tensor_tensor(s2[:], pre[:, 0:SUB],
                                                   w0, s1[:], op0=MULT, op1=ADD)
                    wc = cv_pool.tile([128, SUB], F32, tag="wc")
                    nc.vector.scalar_tensor_tensor(wc[:], pre[:, 2:SUB + 2],
                                                   w2, s2[:], op0=MULT, op1=ADD)
                    t_ = cv_pool.tile([128, SUB], F32, tag="t")
                    nc.gpsimd.tensor_mul(t_[:], g[:], wc[:])
                    z = z_pool.tile([128, SUB], BF16, tag="z")
                    nc.gpsimd.tensor_add(z[:], t_[:], ysb[:])
                    z_tiles.append(z)

                # out-proj + residual
                for r in range(4):
                    xres = xr_pool.tile([128, D], F32, tag="xr")
                    nc.sync.dma_start(
                        xres[:], X[s * SUB + r * 128:s * SUB + (r + 1) * 128, :])
                    osb = o_pool.tile([128, D], F32, tag="osb")
                    for dch in range(2):
                        po = o_ps.tile([128, 512], F32, tag="ops")
                        for ei in range(NET):
                            nc.tensor.matmul(
                                po[:],
                                z_tiles[ei][:, r * 128:(r + 1) * 128],
                                wo_sb[:, ei * D + dch * 512:
                                      ei * D + (dch + 1) * 512],
                                start=(ei == 0), stop=(ei == NET - 1))
                        nc.vector.tensor_add(
                            osb[:, dch * 512:(dch + 1) * 512], po[:],
                            xres[:, dch * 512:(dch + 1) * 512])
                    nc.sync.dma_start(
                        OUT[s * SUB + r * 128:s * SUB + (r + 1) * 128, :],
                        osb[:])
    nc.compile()
    return nc


def prep_inputs(x, A, Bm, Cm, Dv, W_dt, conv_w, W_in, W_out):
    """Host-side folding + per-core sharding. Returns in_maps list."""
    x = np.asarray(x, np.float32)
    A = np.asarray(A, np.float32)
    Bm = np.asarray(Bm, np.float32)
    Cm = np.asarray(Cm, np.float32)
    Dv = np.asarray(Dv, np.float32)
    W_dt = np.asarray(W_dt, np.float32)
    conv_w = np.asarray(conv_w, np.float32)
    W_in = np.asarray(W_in, np.float32)
    W_out = np.asarray(W_out, np.float32)

    W_conv = W_in[:ED]
    W_ssm = W_in[ED:]
    WT = np.ascontiguousarray(
        np.concatenate([W_conv, W_ssm * Dv[:, None]], axis=0).T)  # [D, 2ED]
    w_mean = W_ssm.mean(axis=0, dtype=np.float64).astype(np.float32)  # [D]
    G = (W_ssm.T.astype(np.float64) @ (W_dt[:, 0:1] * Bm).astype(np.float64)
         ).astype(np.float32)                                     # [D, N]
    HM = np.ascontiguousarray(
        np.concatenate([G, w_mean[:, None]], axis=1))             # [D, 17]
    s_a = (A.T.astype(np.float64) @ W_dt[:, 0].astype(np.float64)
           ).astype(np.float32)[:, None]                          # [N, 1]
    WO = np.ascontiguousarray(W_out.T).astype(ml_dtypes.bfloat16)  # [ED, D]
    CMb = np.ascontiguousarray(Cm).astype(ml_dtypes.bfloat16)      # [N, ED]
    CW = np.ascontiguousarray(conv_w[:, 0, :].reshape(NET, 128, 3))
    ident = np.eye(128, dtype=np.float32)

    x_flat = np.ascontiguousarray(x.reshape(B_SZ * L, D))
    in_maps = []
    for c in range(N_CORES):
        b, h = c // 2, c % 2
        g0 = b * L + h * RPC
        xs = x_flat[g0:g0 + RPC]
        if h == 1:
            xw = x_flat[g0 - WARM:g0]
        else:
            xw = np.zeros((WARM, D), np.float32)
        xh = np.zeros((8, D), np.float32)
        for j, rel in enumerate(HALO_REL):
            gr = g0 + rel
            if (h == 0 and rel < 0) or (h == 1 and rel >= RPC):
                continue  # out of batch -> zero pad
            xh[j] = x_flat[gr]
        in_maps.append({
            "x": np.ascontiguousarray(xs), "xw": np.ascontiguousarray(xw),
            "xh": xh, "wt": WT, "wo": WO, "cm": CMb, "hm": HM,
            "sa": s_a, "cw": CW, "ident": ident,
        })
    return in_maps


def kernel(**inputs):
    global _CACHED_NC
    if _CACHED_NC is None:
        _CACHED_NC = build_kernel()
    nc = _CACHED_NC
    in_maps = prep_inputs(**inputs)
    res = run_bass_kernel_spmd(nc, in_maps, list(range(N_CORES)))
    out = np.empty((B_SZ, L, D), np.float32)
    for c in range(N_CORES):
        b, h = c // 2, c % 2
        out[b, h * RPC:(h + 1) * RPC] = res.results[c]["out"]
    return out


# revision 18
# speedup vs baseline: 412.1749x; 412.1749x over previous
"""Trainium2 Bass kernel for nn_DSSM (Mamba-like selective-scan block).

Reference math (B=4, L=4096, D=1024, ED=2048, N=16, K=3):
    proj = x @ W_in.T ; x_conv_pre, x_ssm = split(proj)
    x_conv = depthwise_conv1d(x_conv_pre, conv_w, pad=1)
    dt = mean_e(x_ssm); dtv = dt * W_dt[:,0]
    a = dtv @ A ; u = (dtv * x_ssm) @ Bm          # [b, l, N]
    m_t = a_t * m_{t-1} + u_t  (scan over l)
    y = m @ Cm + Dv * x_ssm
    z = x_conv * sig(y) + y * (1 - sig(y))
    out = z @ W_out.T + x

Algebraic folding (host, exact):
    dt = x @ w_mean              where w_mean = mean_e(W_ssm)
    a  = dt * s_a                where s_a = A.T @ W_dt[:,0]
    u  = dt * (x @ G)            where G = W_ssm.T @ (W_dt[:,0] * Bm)
    Dv folded into the ssm half of W_in (y = m@Cm + x@(Dv*W_ssm).T)

Sharding: core c -> batch c//2, L-half c%2 (2048 rows each). The scan is
seeded by a 512-row warmup for odd cores (max |a| = 0.54 empirically, so
the recurrence forgets its initial state within ~50 steps). Conv boundary
columns come from a small standalone matmul pass (psum [e, 8] layout).

Precision: in-proj / small GEMMs in float32r (TF32-like, ~1.5e-4),
out-proj in bf16 (z and W_out bf16), everything else fp32.
"""
import sys
sys.path.insert(0, '/opt/trn_rl_repo')

import numpy as np
import ml_dtypes

import concourse.bass as bass
import concourse.bacc as bacc
import concourse.tile as tile
import concourse.mybir as mybir
from concourse.bass_utils import run_bass_kernel_spmd

F32 = mybir.dt.float32
F32R = mybir.dt.float32r
BF16 = mybir.dt.bfloat16
MULT = mybir.AluOpType.mult
ADD = mybir.AluOpType.add
SUBT = mybir.AluOpType.subtract
SIG = mybir.ActivationFunctionType.Sigmoid

B_SZ, L, D, ED, N = 4, 4096, 1024, 2048, 16
N_CORES = 8
RPC = 2048          # rows per core
SUB = 512           # rows per sub-chunk
NSUB = RPC // SUB   # 4
WARM = 512          # scan warmup rows (one sub-chunk)
NKT = D // 128      # 8 k-tiles over the contraction dim
NET = ED // 128     # 16 e-tiles per half
EBLK = 2            # e-tiles per weight-strip block

# conv halo row indices relative to the core's first row: head/tail of each
# sub-chunk boundary. head(s) = HALO_HEAD[s], tail(s) = HALO_TAIL[s].
HALO_REL = [-1, 511, 512, 1023, 1024, 1535, 1536, 2048]
HALO_HEAD = [0, 1, 3, 5]
HALO_TAIL = [2, 4, 6, 7]

_CACHED_NC = None


def build_kernel(reps=1):
    nc = bacc.Bacc("TRN2", target_bir_lowering=False, debug=False,
                   num_devices=N_CORES)

    X = nc.dram_tensor("x", [RPC, D], F32, kind="ExternalInput")
    XW = nc.dram_tensor("xw", [WARM, D], F32, kind="ExternalInput")
    XH = nc.dram_tensor("xh", [8, D], F32, kind="ExternalInput")
    WT = nc.dram_tensor("wt", [D, 2 * ED], F32R, kind="ExternalInput")
    WO = nc.dram_tensor("wo", [ED, D], BF16, kind="ExternalInput")
    CM = nc.dram_tensor("cm", [N, ED], BF16, kind="ExternalInput")
    HM = nc.dram_tensor("hm", [D, 17], F32R, kind="ExternalInput")
    SA = nc.dram_tensor("sa", [N, 1], F32, kind="ExternalInput")
    CW = nc.dram_tensor("cw", [NET, 128, 3], F32, kind="ExternalInput")
    IDENT = nc.dram_tensor("ident", [128, 128], F32, kind="ExternalInput")
    OUT = nc.dram_tensor("out", [RPC, D], F32, kind="ExternalOutput")

    with tile.TileContext(nc) as tc:
        with (
            tc.tile_pool(name="const", bufs=1) as cpool,
            tc.tile_pool(name="xnat", bufs=5) as xnat_pool,
            tc.tile_pool(name="xt", bufs=10) as xt_pool,
            tc.tile_pool(name="wst", bufs=20) as w_pool,
            tc.tile_pool(name="pre", bufs=4) as pre_pool,
            tc.tile_pool(name="gy", bufs=3) as gy_pool,
            tc.tile_pool(name="cvt", bufs=3) as cv_pool,
            tc.tile_pool(name="zp", bufs=18) as z_pool,
            tc.tile_pool(name="scn", bufs=2) as s_pool,
            tc.tile_pool(name="ob", bufs=3) as o_pool,
            tc.tile_pool(name="xr", bufs=3) as xr_pool,
            tc.tile_pool(name="tps", bufs=2, space="PSUM") as t_ps,
            tc.tile_pool(name="sps", bufs=1, space="PSUM") as s_ps,
            tc.tile_pool(name="fps", bufs=3, space="PSUM") as f_ps,
            tc.tile_pool(name="ops", bufs=2, space="PSUM") as o_ps,
        ):
            # ---- resident constants (needed early) ----
            ident = cpool.tile([128, 128], F32, tag="ident")
            nc.sync.dma_start(ident[:], IDENT[:])
            h_sb = cpool.tile([128, NKT * 17], F32R, tag="hm")
            nc.sync.dma_start(
                h_sb[:].rearrange("p (k j) -> p k j", k=NKT),
                HM[:].rearrange("(k p) j -> p k j", p=128))
            sa_sb = cpool.tile([N, 1], F32, tag="sa")
            nc.sync.dma_start(sa_sb[:], SA[:])
            halo_all = cpool.tile([128, NET * 8], F32, tag="halo")
            wo_sb = cpool.tile([128, NET * D], BF16, tag="wo")
            cm_sb = cpool.tile([N, ED], BF16, tag="cm")
            cw_sb = cpool.tile([128, NET * 3], F32, tag="cw")
            ones1 = cpool.tile([1, N], F32, tag="ones1")
            nc.vector.memset(ones1[:], 1.0)
            zero16 = cpool.tile([N, 1], F32, tag="zero16")
            nc.vector.memset(zero16[:], 0.0)

            def load_w_strip(k, e0, width, eng):
                wt_t = w_pool.tile([128, EBLK * 128], F32R, tag="wt")
                eng.dma_start(
                    wt_t[:, 0:width],
                    WT[k * 128:(k + 1) * 128, e0 * 128:e0 * 128 + width])
                return wt_t

            def transpose_128(dst, src_ap, pcount=128):
                """PE-transpose src_ap [p, 128] -> dst [128, p] (f32r out)."""
                pt = t_ps.tile([128, pcount], F32, tag="tps")
                nc.tensor.transpose(pt[:], src_ap, ident[0:pcount, 0:pcount])
                nc.scalar.copy(dst, pt[:])

            def halo_prologue():
                xh_nat = xnat_pool.tile([8, D], F32, tag="xnat")
                nc.sync.dma_start(xh_nat[:], XH[:])
                xth = []
                for k in range(NKT):
                    t = cpool.tile([128, 8], F32R, tag=f"xth{k}")
                    transpose_128(t[:], xh_nat[:, k * 128:(k + 1) * 128],
                                  pcount=8)
                    xth.append(t)
                for blk in range(NET // EBLK):
                    strips = [load_w_strip(k, blk * EBLK, EBLK * 128, nc.gpsimd)
                              for k in range(NKT)]
                    for j in range(EBLK):
                        i = blk * EBLK + j
                        ph = s_ps.tile([128, 8], F32, tag="sps")
                        for k in range(NKT):
                            nc.tensor.matmul(
                                ph[:], strips[k][:, j * 128:(j + 1) * 128],
                                xth[k][:], start=(k == 0), stop=(k == NKT - 1))
                        nc.vector.tensor_copy(
                            halo_all[:, i * 8:(i + 1) * 8], ph[:])

            prev_m = None

            def scan_path(xt_tiles, first):
                """small GEMM -> dt broadcast -> a,u -> scan. Returns m tile."""
                nonlocal prev_m
                psv = s_ps.tile([N, SUB], F32, tag="sps")
                for k in range(NKT):
                    nc.tensor.matmul(psv[:], h_sb[:, k * 17:k * 17 + 16],
                                     xt_tiles[k][:], start=(k == 0),
                                     stop=(k == NKT - 1))
                sv = s_pool.tile([N, SUB], F32, tag="sv")
                nc.vector.tensor_copy(sv[:], psv[:])
                pdt = s_ps.tile([1, SUB], F32, tag="sps")
                for k in range(NKT):
                    nc.tensor.matmul(pdt[:], h_sb[:, k * 17 + 16:k * 17 + 17],
                                     xt_tiles[k][:], start=(k == 0),
                                     stop=(k == NKT - 1))
                dtr = s_pool.tile([1, SUB], F32, tag="dtr")
                nc.vector.tensor_copy(dtr[:], pdt[:])
                pdtb = s_ps.tile([N, SUB], F32, tag="sps")
                nc.tensor.matmul(pdtb[:], ones1[:], dtr[:], start=True, stop=True)
                a_sb = s_pool.tile([N, SUB], F32, tag="a")
                nc.vector.tensor_scalar_mul(a_sb[:], pdtb[:], sa_sb[:])
                u_sb = s_pool.tile([N, SUB], F32, tag="u")
                nc.vector.tensor_mul(u_sb[:], sv[:], pdtb[:])
                m = s_pool.tile([N, SUB], F32, tag="m")
                init = zero16[:] if first else prev_m[:, SUB - 1:SUB]
                nc.vector.tensor_tensor_scan(m[:], a_sb[:], u_sb[:], init,
                                             op0=MULT, op1=ADD)
                prev_m = m
                return m

            def load_and_transpose(dram, row0):
                """4 nat tiles + 8 xT tiles [128, SUB] (f32r) for 512 rows."""
                xts = []
                nats = []
                for r in range(4):
                    nat = xnat_pool.tile([128, D], F32, tag="xnat")
                    nc.sync.dma_start(nat[:], dram[row0 + r * 128:
                                                   row0 + (r + 1) * 128, :])
                    nats.append(nat)
                for k in range(NKT):
                    xt = xt_pool.tile([128, SUB], F32R, tag="xt")
                    for r in range(4):
                        transpose_128(xt[:, r * 128:(r + 1) * 128],
                                      nats[r][:, k * 128:(k + 1) * 128])
                    xts.append(xt)
                return xts

            def emit_body(first_rep):
                # warmup sub-chunk (scan only)
                xtw = load_and_transpose(XW, 0)
                scan_path(xtw, first=True)
                halo_prologue()
                if first_rep:
                    # late-needed residents: emit after the warm path so the
                    # startup DMAs prioritize the critical chain
                    nc.scalar.dma_start(cm_sb[:], CM[:])
                    nc.scalar.dma_start(
                        cw_sb[:].rearrange("p (i k) -> p i k", i=NET),
                        CW[:].rearrange("i p k -> p i k"))
                    for ei in range(NET):
                        nc.scalar.dma_start(
                            wo_sb[:, ei * D:(ei + 1) * D],
                            WO[ei * 128:(ei + 1) * 128, :])

                for s in range(NSUB):
                    xts = load_and_transpose(X, s * SUB)
                    m = scan_path(xts, first=False)
                    m_bf = s_pool.tile([N, SUB], BF16, tag="mbf")
                    nc.vector.tensor_copy(m_bf[:], m[:])

                    z_tiles = []
                    for blk in range(NET // EBLK):
                        cstr = [load_w_strip(k, blk * EBLK, EBLK * 128,
                                             nc.gpsimd)
                                for k in range(NKT)]
                        sstr = [load_w_strip(k, NET + blk * EBLK, EBLK * 128,
                                             nc.scalar)
                                for k in range(NKT)]
                        for j in range(EBLK):
                            i = blk * EBLK + j
                            jj = slice(j * 128, (j + 1) * 128)
                            # conv half e-tile
                            pc = f_ps.tile([128, SUB], F32, tag="fps")
                            for k in range(NKT):
                                nc.tensor.matmul(pc[:], cstr[k][:, jj],
                                                 xts[k][:], start=(k == 0),
                                                 stop=(k == NKT - 1))
                            pre = pre_pool.tile([128, SUB + 2], F32, tag="pre")
                            nc.scalar.copy(pre[:, 1:SUB + 1], pc[:])
                            hc = i * 8 + HALO_HEAD[s]
                            tc_ = i * 8 + HALO_TAIL[s]
                            nc.vector.tensor_copy(pre[:, 0:1],
                                                  halo_all[:, hc:hc + 1])
                            nc.vector.tensor_copy(pre[:, SUB + 1:SUB + 2],
                                                  halo_all[:, tc_:tc_ + 1])
                            # ssm half e-tile (+ y accumulation)
                            py = f_ps.tile([128, SUB], F32, tag="fps")
                            for k in range(NKT):
                                nc.tensor.matmul(py[:], sstr[k][:, jj],
                                                 xts[k][:], start=(k == 0),
                                                 stop=False)
                            nc.tensor.matmul(py[:],
                                             cm_sb[:, i * 128:(i + 1) * 128],
                                             m_bf[:], start=False, stop=True)
                            g = gy_pool.tile([128, SUB], F32, tag="g")
                            nc.scalar.activation(g[:], py[:], SIG)
                            ysb = gy_pool.tile([128, SUB], F32, tag="ysb")
                            nc.vector.tensor_copy(ysb[:], py[:])
                            # conv + gate: w = conv(pre) - y ; z = y + sig(y)*w
                            w0 = cw_sb[:, i * 3 + 0:i * 3 + 1]
                            w1 = cw_sb[:, i * 3 + 1:i * 3 + 2]
                            w2 = cw_sb[:, i * 3 + 2:i * 3 + 3]
                            s1 = cv_pool.tile([128, SUB], F32, tag="s1")
                            nc.vector.scalar_tensor_tensor(
                                s1[:], pre[:, 1:SUB + 1], w1, ysb[:],
                                op0=MULT, op1=SUBT)
                            s2 = cv_pool.tile([128, SUB], F32, tag="s2")
                            nc.vector.scalar_tensor_tensor(
                                s2[:], pre[:, 0:SUB], w0, s1[:],
                                op0=MULT, op1=ADD)
                            wc = cv_pool.tile([128, SUB], F32, tag="wc")
                            nc.vector.scalar_tensor_tensor(
                                wc[:], pre[:, 2:SUB + 2], w2, s2[:],
                                op0=MULT, op1=ADD)
                            t_ = cv_pool.tile([128, SUB], F32, tag="t")
                            nc.gpsimd.tensor_mul(t_[:], g[:], wc[:])
                            z = z_pool.tile([128, SUB], BF16, tag="z")
                            nc.gpsimd.tensor_add(z[:], t_[:], ysb[:])
                            z_tiles.append(z)

                    # out-proj + residual
                    for r in range(4):
                        xres = xr_pool.tile([128, D], F32, tag="xr")
                        nc.sync.dma_start(
                            xres[:],
                            X[s * SUB + r * 128:s * SUB + (r + 1) * 128, :])
                        osb = o_pool.tile([128, D], F32, tag="osb")
                        for dch in range(2):
                            po = o_ps.tile([128, 512], F32, tag="ops")
                            for ei in range(NET):
                                nc.tensor.matmul(
                                    po[:],
                                    z_tiles[ei][:, r * 128:(r + 1) * 128],
                                    wo_sb[:, ei * D + dch * 512:
                                          ei * D + (dch + 1) * 512],
                                    start=(ei == 0), stop=(ei == NET - 1))
                            nc.vector.tensor_add(
                                osb[:, dch * 512:(dch + 1) * 512], po[:],
                                xres[:, dch * 512:(dch + 1) * 512])
                        nc.sync.dma_start(
                            OUT[s * SUB + r * 128:s * SUB + (r + 1) * 128, :],
                            osb[:])

            for rep in range(reps):
                emit_body(rep == 0)
    nc.compile()
    return nc


def prep_inputs(x, A, Bm, Cm, Dv, W_dt, conv_w, W_in, W_out):
    """Host-side folding + per-core sharding. Returns in_maps list."""
    x = np.asarray(x, np.float32)
    A = np.asarray(A, np.float32)
    Bm = np.asarray(Bm, np.float32)
    Cm = np.asarray(Cm, np.float32)
    Dv = np.asarray(Dv, np.float32)
    W_dt = np.asarray(W_dt, np.float32)
    conv_w = np.asarray(conv_w, np.float32)
    W_in = np.asarray(W_in, np.float32)
    W_out = np.asarray(W_out, np.float32)

    W_conv = W_in[:ED]
    W_ssm = W_in[ED:]
    WT = np.ascontiguousarray(
        np.concatenate([W_conv, W_ssm * Dv[:, None]], axis=0).T)  # [D, 2ED]
    w_mean = W_ssm.mean(axis=0, dtype=np.float64).astype(np.float32)  # [D]
    G = (W_ssm.T.astype(np.float64) @ (W_dt[:, 0:1] * Bm).astype(np.float64)
         ).astype(np.float32)                                     # [D, N]
    HM = np.ascontiguousarray(
        np.concatenate([G, w_mean[:, None]], axis=1))             # [D, 17]
    s_a = (A.T.astype(np.float64) @ W_dt[:, 0].astype(np.float64)
           ).astype(np.float32)[:, None]                          # [N, 1]
    WO = np.ascontiguousarray(W_out.T).astype(ml_dtypes.bfloat16)  # [ED, D]
    CMb = np.ascontiguousarray(Cm).astype(ml_dtypes.bfloat16)      # [N, ED]
    CW = np.ascontiguousarray(conv_w[:, 0, :].reshape(NET, 128, 3))
    ident = np.eye(128, dtype=np.float32)

    x_flat = np.ascontiguousarray(x.reshape(B_SZ * L, D))
    in_maps = []
    for c in range(N_CORES):
        b, h = c // 2, c % 2
        g0 = b * L + h * RPC
        xs = x_flat[g0:g0 + RPC]
        if h == 1:
            xw = x_flat[g0 - WARM:g0]
        else:
            xw = np.zeros((WARM, D), np.float32)
        xh = np.zeros((8, D), np.float32)
        for j, rel in enumerate(HALO_REL):
            gr = g0 + rel
            if (h == 0 and rel < 0) or (h == 1 and rel >= RPC):
                continue  # out of batch -> zero pad
            xh[j] = x_flat[gr]
        in_maps.append({
            "x": np.ascontiguousarray(xs), "xw": np.ascontiguousarray(xw),
            "xh": xh, "wt": WT, "wo": WO, "cm": CMb, "hm": HM,
            "sa": s_a, "cw": CW, "ident": ident,
        })
    return in_maps


def kernel(**inputs):
    global _CACHED_NC
    if _CACHED_NC is None:
        _CACHED_NC = build_kernel()
    nc = _CACHED_NC
    in_maps = prep_inputs(**inputs)
    res = run_bass_kernel_spmd(nc, in_maps, list(range(N_CORES)))
    out = np.empty((B_SZ, L, D), np.float32)
    for c in range(N_CORES):
        b, h = c // 2, c % 2
        out[b, h * RPC:(h + 1) * RPC] = res.results[c]["out"]
    return out


# revision 50
# speedup vs baseline: 543.2352x; 1.3180x over previous
"""Trainium2 Bass kernel for nn_DSSM (Mamba-like selective-scan block).

Reference math (B=4, L=4096, D=1024, ED=2048, N=16, K=3):
    proj = x @ W_in.T ; x_conv_pre, x_ssm = split(proj)
    x_conv = depthwise_conv1d(x_conv_pre, conv_w, pad=1)
    dt = mean_e(x_ssm); dtv = dt * W_dt[:,0]
    a = dtv @ A ; u = (dtv * x_ssm) @ Bm          # [b, l, N]
    m_t = a_t * m_{t-1} + u_t  (scan over l)
    y = m @ Cm + Dv * x_ssm
    z = x_conv * sig(y) + y * (1 - sig(y))
    out = z @ W_out.T + x

Algebraic folding (host, exact):
    dt = x @ w_mean              where w_mean = mean_e(W_ssm)
    a  = dt * s_a                where s_a = A.T @ W_dt[:,0]
    u  = dt * (x @ G)            where G = W_ssm.T @ (W_dt[:,0] * Bm)
    Dv folded into the ssm half of W_in (y = m@Cm + x@(Dv*W_ssm).T)

Sharding: core c -> batch c//2, L-half c%2 (2048 rows each). The scan is
seeded by a 512-row warmup for odd cores (max |a| = 0.54 empirically, so
the recurrence forgets its initial state within ~50 steps). Conv boundary
columns come from a small standalone matmul pass (psum [e, 8] layout).

Precision: in-proj / small GEMMs in float32r (TF32-like, ~1.5e-4),
out-proj in bf16 (z and W_out bf16), everything else fp32.
"""
import sys
sys.path.insert(0, '/opt/trn_rl_repo')

import numpy as np
import ml_dtypes

import concourse.bass as bass
import concourse.bacc as bacc
import concourse.tile as tile
import concourse.mybir as mybir
from concourse.bass_utils import run_bass_kernel_spmd

F32 = mybir.dt.float32
F32R = mybir.dt.float32r
BF16 = mybir.dt.bfloat16
MULT = mybir.AluOpType.mult
ADD = mybir.AluOpType.add
SUBT = mybir.AluOpType.subtract
SIG = mybir.ActivationFunctionType.Sigmoid

B_SZ, L, D, ED, N = 4, 4096, 1024, 2048, 16
N_CORES = 8
RPC = 2048          # rows per core
SUB = 512           # rows per sub-chunk
NSUB = RPC // SUB   # 4
WARM = 128          # scan warmup rows (max |a| = 0.54 -> leak ~1e-34)
NKT = D // 128      # 8 k-tiles over the contraction dim
NET = ED // 128     # 16 e-tiles per half
EBLK = 2            # e-tiles per weight-strip block

# conv halo row indices relative to the core's first row: head/tail of each
# sub-chunk boundary. head(s) = HALO_HEAD[s], tail(s) = HALO_TAIL[s].
HALO_REL = [-1, 511, 512, 1023, 1024, 1535, 1536, 2048]
HALO_HEAD = [0, 1, 3, 5]
HALO_TAIL = [2, 4, 6, 7]

_CACHED_NC = None


def build_kernel(reps=1):
    nc = bacc.Bacc("TRN2", target_bir_lowering=False, debug=False,
                   num_devices=N_CORES)

    X = nc.dram_tensor("x", [RPC, D], F32, kind="ExternalInput")
    XT = nc.dram_tensor("xt", [D, RPC], F32R, kind="ExternalInput")
    XWT = nc.dram_tensor("xwt", [D, WARM], F32R, kind="ExternalInput")
    XHT = nc.dram_tensor("xht", [D, 8], F32R, kind="ExternalInput")
    WT = nc.dram_tensor("wt", [D, 2 * ED], F32R, kind="ExternalInput")
    WO = nc.dram_tensor("wo", [ED, D], BF16, kind="ExternalInput")
    CM = nc.dram_tensor("cm", [N, ED], F32R, kind="ExternalInput")
    HM = nc.dram_tensor("hm", [D, 17], F32R, kind="ExternalInput")
    SA = nc.dram_tensor("sa", [N, 1], F32, kind="ExternalInput")
    CW = nc.dram_tensor("cw", [NET, 128, 3], F32, kind="ExternalInput")
    OUT = nc.dram_tensor("out", [RPC, D], F32, kind="ExternalOutput")

    with tile.TileContext(nc) as tc:
        with (
            tc.tile_pool(name="const", bufs=1) as cpool,
            tc.tile_pool(name="xt", bufs=12) as xt_pool,
            tc.tile_pool(name="wst", bufs=20) as w_pool,
            tc.tile_pool(name="pre", bufs=4) as pre_pool,
            tc.tile_pool(name="gy", bufs=4) as gy_pool,
            tc.tile_pool(name="cvt", bufs=3) as cv_pool,
            tc.tile_pool(name="zp", bufs=18) as z_pool,
            tc.tile_pool(name="scn", bufs=2) as s_pool,
            tc.tile_pool(name="ob", bufs=3) as o_pool,
            tc.tile_pool(name="xr", bufs=3) as xr_pool,
            tc.tile_pool(name="sps", bufs=2, space="PSUM") as s_ps,
            tc.tile_pool(name="fps", bufs=3, space="PSUM") as f_ps,
            tc.tile_pool(name="ops", bufs=3, space="PSUM") as o_ps,
        ):
            # ---- resident constants (needed early) ----
            h_sb = cpool.tile([128, NKT * 17], F32R, tag="hm")
            nc.sync.dma_start(
                h_sb[:].rearrange("p (k j) -> p k j", k=NKT),
                HM[:].rearrange("(k p) j -> p k j", p=128))
            sa_sb = cpool.tile([N, 1], F32, tag="sa")
            nc.sync.dma_start(sa_sb[:], SA[:])
            halo_all = cpool.tile([128, NET * 8], F32, tag="halo")
            wo_sb = cpool.tile([128, NET * D], BF16, tag="wo")
            cm_sb = cpool.tile([N, ED], F32R, tag="cm")
            cw_sb = cpool.tile([128, NET * 3], F32, tag="cw")
            ones1 = cpool.tile([1, N], F32, tag="ones1")
            nc.vector.memset(ones1[:], 1.0)
            zero16 = cpool.tile([N, 1], F32, tag="zero16")
            nc.vector.memset(zero16[:], 0.0)

            def load_w_strip(k, e0, width, eng):
                wt_t = w_pool.tile([128, EBLK * 128], F32R, tag="wt")
                eng.dma_start(
                    wt_t[:, 0:width],
                    WT[k * 128:(k + 1) * 128, e0 * 128:e0 * 128 + width])
                return wt_t

            def halo_loads():
                """xth tiles [128, 8] (f32r) from the 8 halo rows."""
                xth = []
                for k in range(NKT):
                    t = cpool.tile([128, 8], F32R, tag=f"xth{k}")
                    nc.sync.dma_start(t[:], XHT[k * 128:(k + 1) * 128, :])
                    xth.append(t)
                return xth

            prev_m = [None, 0]   # tile, width
            xth_cell = []

            def scan_path(xt_tiles, first, width=SUB):
                """small GEMM -> dt broadcast -> a,u -> scan. Returns m tile."""
                psv = s_ps.tile([N, width], F32, tag="sps")
                for k in range(NKT):
                    nc.tensor.matmul(psv[:], h_sb[:, k * 17:k * 17 + 16],
                                     xt_tiles[k][:], start=(k == 0),
                                     stop=(k == NKT - 1))
                sv = s_pool.tile([N, width], F32, tag="sv")
                nc.vector.tensor_copy(sv[:], psv[:])
                pdt = s_ps.tile([1, width], F32, tag="sps")
                for k in range(NKT):
                    nc.tensor.matmul(pdt[:], h_sb[:, k * 17 + 16:k * 17 + 17],
                                     xt_tiles[k][:], start=(k == 0),
                                     stop=(k == NKT - 1))
                dtr = s_pool.tile([1, width], F32, tag="dtr")
                nc.vector.tensor_copy(dtr[:], pdt[:])
                pdtb = s_ps.tile([N, width], F32, tag="sps")
                nc.tensor.matmul(pdtb[:], ones1[:], dtr[:], start=True, stop=True)
                a_sb = s_pool.tile([N, width], F32, tag="a")
                nc.vector.tensor_scalar_mul(a_sb[:], pdtb[:], sa_sb[:])
                u_sb = s_pool.tile([N, width], F32, tag="u")
                nc.vector.tensor_mul(u_sb[:], sv[:], pdtb[:])
                m = s_pool.tile([N, width], F32, tag="m")
                if first:
                    init = zero16[:]
                else:
                    pm, pw = prev_m
                    init = pm[:, pw - 1:pw]
                nc.vector.tensor_tensor_scan(m[:], a_sb[:], u_sb[:], init,
                                             op0=MULT, op1=ADD)
                prev_m[0] = m
                prev_m[1] = width
                return m

            def load_xt(dram, row0, width=SUB):
                """8 xT tiles [128, width] (f32r) from host-transposed x."""
                xts = []
                tag = "xt" if width == SUB else "xtw"
                for k in range(NKT):
                    xt = xt_pool.tile([128, width], F32R, tag=tag)
                    nc.sync.dma_start(
                        xt[:], dram[k * 128:(k + 1) * 128, row0:row0 + width])
                    xts.append(xt)
                return xts

            def emit_body(first_rep):
                # prime sub 0's loads first, then the warm scan
                xts0 = load_xt(XT, 0)
                xtw = load_xt(XWT, 0, width=WARM)
                scan_path(xtw, first=True, width=WARM)
                if first_rep:
                    xth_cell.extend(halo_loads())
                xth = xth_cell
                if first_rep:
                    nc.sync.dma_start(cm_sb[:], CM[:])
                    nc.sync.dma_start(
                        cw_sb[:].rearrange("p (i k) -> p i k", i=NET),
                        CW[:].rearrange("i p k -> p i k"))

                for s in range(NSUB):
                    xts = xts0 if s == 0 else load_xt(XT, s * SUB)
                    m = scan_path(xts, first=False)
                    m_bf = s_pool.tile([N, SUB], F32R, tag="mbf")
                    nc.scalar.copy(m_bf[:], m[:])

                    z_tiles = []
                    for blk in range(NET // EBLK):
                        cstr = [load_w_strip(k, blk * EBLK, EBLK * 128,
                                             nc.gpsimd)
                                for k in range(NKT)]
                        sstr = [load_w_strip(k, NET + blk * EBLK, EBLK * 128,
                                             nc.sync)
                                for k in range(NKT)]
                        for j in range(EBLK):
                            i = blk * EBLK + j
                            jj = slice(j * 128, (j + 1) * 128)
                            # conv half e-tile
                            pc = f_ps.tile([128, SUB], F32, tag="fps")
                            for k in range(NKT):
                                nc.tensor.matmul(pc[:], cstr[k][:, jj],
                                                 xts[k][:], start=(k == 0),
                                                 stop=(k == NKT - 1))
                            if s == 0:
                                # conv halo rows ride sub 0's weight strips
                                ph = s_ps.tile([128, 8], F32, tag="sps")
                                for k in range(NKT):
                                    nc.tensor.matmul(
                                        ph[:], cstr[k][:, jj], xth[k][:],
                                        start=(k == 0), stop=(k == NKT - 1))
                                nc.vector.tensor_copy(
                                    halo_all[:, i * 8:(i + 1) * 8], ph[:])
                            pre = pre_pool.tile([128, SUB + 2], F32, tag="pre")
                            nc.scalar.copy(pre[:, 1:SUB + 1], pc[:])
                            hc = i * 8 + HALO_HEAD[s]
                            tc_ = i * 8 + HALO_TAIL[s]
                            nc.vector.tensor_copy(pre[:, 0:1],
                                                  halo_all[:, hc:hc + 1])
                            nc.vector.tensor_copy(pre[:, SUB + 1:SUB + 2],
                                                  halo_all[:, tc_:tc_ + 1])
                            # ssm half e-tile (+ y accumulation)
                            py = f_ps.tile([128, SUB], F32, tag="fps")
                            for k in range(NKT):
                                nc.tensor.matmul(py[:], sstr[k][:, jj],
                                                 xts[k][:], start=(k == 0),
                                                 stop=False)
                            nc.tensor.matmul(py[:],
                                             cm_sb[:, i * 128:(i + 1) * 128],
                                             m_bf[:], start=False, stop=True)
                            g = gy_pool.tile([128, SUB], F32, tag="g")
                            nc.scalar.activation(g[:], py[:], SIG)
                            ysb = gy_pool.tile([128, SUB], F32, tag="ysb")
                            nc.vector.tensor_copy(ysb[:], py[:])
                            # conv + gate: w = conv(pre) - y ; z = y + sig(y)*w
                            w0 = cw_sb[:, i * 3 + 0:i * 3 + 1]
                            w1 = cw_sb[:, i * 3 + 1:i * 3 + 2]
                            w2 = cw_sb[:, i * 3 + 2:i * 3 + 3]
                            s1 = cv_pool.tile([128, SUB], F32, tag="s1")
                            nc.vector.scalar_tensor_tensor(
                                s1[:], pre[:, 1:SUB + 1], w1, ysb[:],
                                op0=MULT, op1=SUBT)
                            s2 = cv_pool.tile([128, SUB], F32, tag="s2")
                            nc.vector.scalar_tensor_tensor(
                                s2[:], pre[:, 0:SUB], w0, s1[:],
                                op0=MULT, op1=ADD)
                            wc = cv_pool.tile([128, SUB], F32, tag="wc")
                            nc.vector.scalar_tensor_tensor(
                                wc[:], pre[:, 2:SUB + 2], w2, s2[:],
                                op0=MULT, op1=ADD)
                            t_ = cv_pool.tile([128, SUB], F32, tag="t")
                            nc.gpsimd.tensor_mul(t_[:], g[:], wc[:])
                            z = z_pool.tile([128, SUB], BF16, tag="z")
                            nc.gpsimd.tensor_add(z[:], t_[:], ysb[:])
                            z_tiles.append(z)

                    # out-proj + residual
                    if first_rep and s == 0:
                        for ei in range(NET):
                            nc.scalar.dma_start(
                                wo_sb[:, ei * D:(ei + 1) * D],
                                WO[ei * 128:(ei + 1) * 128, :])
                    for r in range(4):
                        xres = xr_pool.tile([128, D], F32, tag="xr")
                        nc.sync.dma_start(
                            xres[:],
                            X[s * SUB + r * 128:s * SUB + (r + 1) * 128, :])
                        osb = o_pool.tile([128, D], F32, tag="osb")
                        for dch in range(2):
                            po = o_ps.tile([128, 512], F32, tag="ops")
                            for ei in range(NET):
                                nc.tensor.matmul(
                                    po[:],
                                    z_tiles[ei][:, r * 128:(r + 1) * 128],
                                    wo_sb[:, ei * D + dch * 512:
                                          ei * D + (dch + 1) * 512],
                                    start=(ei == 0), stop=(ei == NET - 1))
                            nc.vector.tensor_add(
                                osb[:, dch * 512:(dch + 1) * 512], po[:],
                                xres[:, dch * 512:(dch + 1) * 512])
                        nc.sync.dma_start(
                            OUT[s * SUB + r * 128:s * SUB + (r + 1) * 128, :],
                            osb[:])

            for rep in range(reps):
                emit_body(rep == 0)
    nc.compile()
    return nc


def prep_inputs(x, A, Bm, Cm, Dv, W_dt, conv_w, W_in, W_out):
    """Host-side folding + per-core sharding. Returns in_maps list."""
    x = np.asarray(x, np.float32)
    A = np.asarray(A, np.float32)
    Bm = np.asarray(Bm, np.float32)
    Cm = np.asarray(Cm, np.float32)
    Dv = np.asarray(Dv, np.float32)
    W_dt = np.asarray(W_dt, np.float32)
    conv_w = np.asarray(conv_w, np.float32)
    W_in = np.asarray(W_in, np.float32)
    W_out = np.asarray(W_out, np.float32)

    W_conv = W_in[:ED]
    W_ssm = W_in[ED:]
    WT = np.ascontiguousarray(
        np.concatenate([W_conv, W_ssm * Dv[:, None]], axis=0).T)  # [D, 2ED]
    w_mean = W_ssm.mean(axis=0, dtype=np.float64).astype(np.float32)  # [D]
    G = (W_ssm.T.astype(np.float64) @ (W_dt[:, 0:1] * Bm).astype(np.float64)
         ).astype(np.float32)                                     # [D, N]
    HM = np.ascontiguousarray(
        np.concatenate([G, w_mean[:, None]], axis=1))             # [D, 17]
    s_a = (A.T.astype(np.float64) @ W_dt[:, 0].astype(np.float64)
           ).astype(np.float32)[:, None]                          # [N, 1]
    WO = np.ascontiguousarray(W_out.T).astype(ml_dtypes.bfloat16)  # [ED, D]
    CMb = np.ascontiguousarray(Cm)                                 # [N, ED] f32r
    CW = np.ascontiguousarray(conv_w[:, 0, :].reshape(NET, 128, 3))

    x_flat = np.ascontiguousarray(x.reshape(B_SZ * L, D))
    in_maps = []
    for c in range(N_CORES):
        b, h = c // 2, c % 2
        g0 = b * L + h * RPC
        xs = x_flat[g0:g0 + RPC]
        if h == 1:
            xw = x_flat[g0 - WARM:g0]
        else:
            xw = np.zeros((WARM, D), np.float32)
        xh = np.zeros((8, D), np.float32)
        for j, rel in enumerate(HALO_REL):
            gr = g0 + rel
            if (h == 0 and rel < 0) or (h == 1 and rel >= RPC):
                continue  # out of batch -> zero pad
            xh[j] = x_flat[gr]
        in_maps.append({
            "x": np.ascontiguousarray(xs),
            "xt": np.ascontiguousarray(xs.T),
            "xwt": np.ascontiguousarray(xw.T),
            "xht": np.ascontiguousarray(xh.T),
            "wt": WT, "wo": WO, "cm": CMb, "hm": HM,
            "sa": s_a, "cw": CW,
        })
    return in_maps


def kernel(**inputs):
    global _CACHED_NC
    if _CACHED_NC is None:
        _CACHED_NC = build_kernel()
    nc = _CACHED_NC
    in_maps = prep_inputs(**inputs)
    res = run_bass_kernel_spmd(nc, in_maps, list(range(N_CORES)))
    out = np.empty((B_SZ, L, D), np.float32)
    for c in range(N_CORES):
        b, h = c // 2, c % 2
        out[b, h * RPC:(h + 1) * RPC] = res.results[c]["out"]
    return out
